# revision 2
# baseline (speedup 1.0000x reference)
"""Trainium2 Bass kernel for nn_Kalman_filter_34041910788634.

Mathematical collapse of the reference:
  - The scan's step() ignores its carry (st, e_t = inp rebinds both from the
    scan inputs), and the parameter-network output o is time-invariant, so the
    whole T_LEN-step loop reduces to evaluating the last step (T[-1], e[-1]).
  - The second MLP matmul (h @ W2.T, 34 GFLOP) is only consumed through dot
    products with e8 and T8, so it collapses to h @ (W2.T @ e8) and
    h[0] @ (W2.T @ T8): two matvecs.

Remaining device-side work per core k (hidden dim sharded 8 ways):
  hQ_k.T = relu(W1_k @ Q.T + b1_k)          [512, 2048]  (the one big matmul)
  aq_k   = ve_k.T @ hQ_k.T                  [2048]       (partial over hidden)
  hw_k   = relu(w @ W1_k.T + b1_k)          [512]
  qe_k   = Q[k-slice] @ e8                  [256]
Host combines (tiny BLAS-1/2): ve/vT matvecs, p_wst scalar, st, final fc.
"""

import os
import sys

for _p in ("/opt/trn_rl_repo", "/root/.axon_site/_ro/trn_rl_repo"):
    if os.path.isdir(_p) and _p not in sys.path:
        sys.path.insert(0, _p)

import numpy as np

import concourse.bass as bass
import concourse.bass2jax as _bass2jax
import concourse.mybir as mybir
import concourse.tile as tile
from concourse.bass_utils import run_bass_kernel_spmd


def _split_multiwaits(bir_bytes):
    """The walrus build in this container supports at most one sync-wait
    condition per instruction; Tile freely emits several.  Hoist extra waits
    onto NoOp instructions inserted just before the owning instruction (same
    engine, so per-engine program order makes this equivalent)."""
    import orjson

    b = orjson.loads(bir_bytes)
    n = 0
    for func in b.get("functions", []):
        for blk in func.get("blocks", []):
            newl = []
            for ins in blk.get("instructions", []):
                si = ins.get("sync_info")
                ws = (si or {}).get("on_wait") or []
                if len(ws) > 1:
                    for wv in ws[:-1]:
                        n += 1
                        newl.append({
                            "debug": ins.get("debug", 0),
                            "engine": ins["engine"],
                            "ins": [],
                            "outs": [],
                            "name": f"{ins['name']}-wsplit{n}",
                            "opcode": "NoOp",
                            "sync_info": {"on_update": [], "on_wait": [wv]},
                        })
                    si["on_wait"] = ws[-1:]
                newl.append(ins)
            blk["instructions"] = newl
    return orjson.dumps(b)


_orig_compile_bir_kernel = _bass2jax.compile_bir_kernel


def _patched_compile_bir_kernel(ant_bir_str, compile_dir, neff_name="file.neff"):
    return _orig_compile_bir_kernel(
        _split_multiwaits(ant_bir_str), compile_dir, neff_name=neff_name
    )


if _bass2jax.compile_bir_kernel is not _patched_compile_bir_kernel:
    _bass2jax.compile_bir_kernel = _patched_compile_bir_kernel

N_DIM = 2048
HIDDEN = 4096
OUT_DIM = 512
NCORES = 8
JSH = HIDDEN // NCORES      # 512 hidden units per core
RSL = N_DIM // NCORES       # 256 Q rows per core (for qe partial)
DC = N_DIM // 128           # 16 contraction chunks
JC = JSH // 128             # 4 lhsT column chunks
RC = N_DIM // 512           # 4 moving-dim chunks of 512

FR = mybir.dt.float32r
F32 = mybir.dt.float32
RELU = mybir.ActivationFunctionType.Relu

_cache = {}


def _build_nc():
    nc = bass.Bass(target_bir_lowering=False)

    qtr = nc.dram_tensor("qtr", [RC, N_DIM, 512], FR, kind="ExternalInput")
    w1t = nc.dram_tensor("w1t", [N_DIM, JSH], FR, kind="ExternalInput")
    qte = nc.dram_tensor("qte", [N_DIM, RSL], FR, kind="ExternalInput")
    wc = nc.dram_tensor("wc", [128, DC], FR, kind="ExternalInput")
    e8c = nc.dram_tensor("e8c", [128, DC], FR, kind="ExternalInput")
    vec = nc.dram_tensor("vec", [128, JC], FR, kind="ExternalInput")
    b1c = nc.dram_tensor("b1c", [128, JC], F32, kind="ExternalInput")
    b1r = nc.dram_tensor("b1r", [1, JSH], F32, kind="ExternalInput")
    aq = nc.dram_tensor("aq", [1, N_DIM], F32, kind="ExternalOutput")
    qe = nc.dram_tensor("qe", [1, RSL], F32, kind="ExternalOutput")
    hwr = nc.dram_tensor("hwr", [1, JSH], F32, kind="ExternalOutput")

    with tile.TileContext(nc) as tc:
        with (
            tc.tile_pool(name="wpool", bufs=1) as wpool,
            tc.tile_pool(name="qpool", bufs=2) as qpool,
            tc.tile_pool(name="qepool", bufs=1) as qepool,
            tc.tile_pool(name="small", bufs=1) as small,
            tc.tile_pool(name="hpool", bufs=6) as hpool,
            tc.tile_pool(name="opool", bufs=1) as opool,
            tc.tile_pool(name="psh", bufs=4, space="PSUM") as psh,
            tc.tile_pool(name="psv", bufs=2, space="PSUM") as psv,
        ):
            wc_s = small.tile([128, DC], FR, name="wc_s")
            nc.sync.dma_start(wc_s[:], wc[:])
            e8_s = small.tile([128, DC], FR, name="e8_s")
            nc.sync.dma_start(e8_s[:], e8c[:])
            vec_s = small.tile([128, JC], FR, name="vec_s")
            nc.sync.dma_start(vec_s[:], vec[:])
            b1c_s = small.tile([128, JC], F32, name="b1c_s")
            nc.sync.dma_start(b1c_s[:], b1c[:])
            b1r_s = small.tile([1, JSH], F32, name="b1r_s")
            nc.sync.dma_start(b1r_s[:], b1r[:])

            w1ts = []
            for dc in range(DC):
                t = wpool.tile([128, JSH], FR, name=f"w1t_{dc}", tag=f"w1t_{dc}")
                nc.sync.dma_start(t[:], w1t[dc * 128:(dc + 1) * 128, :])
                w1ts.append(t)

            aq_s = opool.tile([1, N_DIM], F32, name="aq_s")

            for rc in range(RC):
                qts = []
                for dc in range(DC):
                    t = qpool.tile([128, 512], FR, name=f"qt_{rc}_{dc}", tag=f"qt_{dc}")
                    nc.sync.dma_start(t[:], qtr[rc, dc * 128:(dc + 1) * 128, :])
                    qts.append(t)
                hrelus = []
                for jc in range(JC):
                    ph = psh.tile([128, 512], F32, name=f"ph_{rc}_{jc}", tag="ph")
                    for dc in range(DC):
                        nc.tensor.matmul(
                            ph[:],
                            w1ts[dc][:, jc * 128:(jc + 1) * 128],
                            qts[dc][:],
                            start=(dc == 0),
                            stop=(dc == DC - 1),
                        )
                    hr = hpool.tile([128, 512], FR, name=f"hr_{rc}_{jc}", tag="hr")
                    nc.scalar.activation(hr[:], ph[:], RELU, bias=b1c_s[:, jc:jc + 1])
                    hrelus.append(hr)
                pa = psv.tile([1, 512], F32, name=f"pa_{rc}", tag="pa")
                for jc in range(JC):
                    nc.tensor.matmul(
                        pa[:],
                        vec_s[:, jc:jc + 1],
                        hrelus[jc][:],
                        start=(jc == 0),
                        stop=(jc == JC - 1),
                    )
                nc.vector.tensor_copy(aq_s[:, rc * 512:(rc + 1) * 512], pa[:])

            nc.sync.dma_start(aq[:], aq_s[:])

            # hw row: relu(w @ W1_k.T + b1_k) with hidden on the free dim
            pw = psv.tile([1, JSH], F32, name="pw", tag="pw", bufs=1)
            for dc in range(DC):
                nc.tensor.matmul(
                    pw[:], wc_s[:, dc:dc + 1], w1ts[dc][:],
                    start=(dc == 0), stop=(dc == DC - 1),
                )
            hw1 = opool.tile([1, JSH], F32, name="hw1")
            nc.vector.tensor_add(hw1[:], pw[:], b1r_s[:])
            hw2 = opool.tile([1, JSH], F32, name="hw2")
            nc.scalar.activation(hw2[:], hw1[:], RELU)
            nc.sync.dma_start(hwr[:], hw2[:])

            # qe partial: Q[k-slice] @ e8 via lhsT = e8 chunks
            qtes = []
            for dc in range(DC):
                t = qepool.tile([128, RSL], FR, name=f"qte_{dc}", tag=f"qte_{dc}")
                nc.sync.dma_start(t[:], qte[dc * 128:(dc + 1) * 128, :])
                qtes.append(t)
            pq = psv.tile([1, RSL], F32, name="pq", tag="pq", bufs=1)
            for dc in range(DC):
                nc.tensor.matmul(
                    pq[:], e8_s[:, dc:dc + 1], qtes[dc][:],
                    start=(dc == 0), stop=(dc == DC - 1),
                )
            qe_s = opool.tile([1, RSL], F32, name="qe_s")
            nc.vector.tensor_copy(qe_s[:], pq[:])
            nc.sync.dma_start(qe[:], qe_s[:])

    return nc


def _get_nc():
    if "nc" not in _cache:
        _cache["nc"] = _build_nc()
    return _cache["nc"]


def _col128(v):
    """[n*128] -> [128, n] with v[c*128+p] at [p, c]."""
    return np.ascontiguousarray(v.reshape(-1, 128).T)


def kernel(**inputs):
    T = np.asarray(inputs["T"], np.float32)
    e = np.asarray(inputs["e"], np.float32)
    w = np.asarray(inputs["w"], np.float32)
    Q = np.asarray(inputs["Q"], np.float32)
    W1 = np.asarray(inputs["W1"], np.float32)
    b1 = np.asarray(inputs["b1"], np.float32)
    W2 = np.asarray(inputs["W2"], np.float32)
    b2 = np.asarray(inputs["b2"], np.float32)
    fc_w = np.asarray(inputs["fc_w"], np.float32)
    fc_b = np.asarray(inputs["fc_b"], np.float32)

    T8 = T[-1]
    e8 = e[-1]

    QT = np.ascontiguousarray(Q.T)                      # [d, r]
    qtr = np.ascontiguousarray(
        QT.reshape(N_DIM, RC, 512).transpose(1, 0, 2))  # [rc, d, 512]
    ve = e8 @ W2                                        # [4096] = W2.T @ e8
    vT = T8 @ W2
    wc = _col128(w)
    e8c = _col128(e8)

    in_maps = []
    for k in range(NCORES):
        in_maps.append({
            "qtr": qtr,
            "w1t": np.ascontiguousarray(W1[k * JSH:(k + 1) * JSH, :].T),
            "qte": np.ascontiguousarray(QT[:, k * RSL:(k + 1) * RSL]),
            "wc": wc,
            "e8c": e8c,
            "vec": _col128(ve[k * JSH:(k + 1) * JSH]),
            "b1c": _col128(b1[k * JSH:(k + 1) * JSH]),
            "b1r": np.ascontiguousarray(b1[k * JSH:(k + 1) * JSH].reshape(1, JSH)),
        })

    res = run_bass_kernel_spmd(_get_nc(), in_maps, core_ids=list(range(NCORES))).results

    aQ = np.zeros(N_DIM, np.float64)
    for k in range(NCORES):
        aQ += res[k]["aq"][0].astype(np.float64)
    Qe = np.concatenate([res[k]["qe"][0] for k in range(NCORES)]).astype(np.float64)
    hw = np.concatenate([res[k]["hwr"][0] for k in range(NCORES)]).astype(np.float64)

    g0 = float(hw @ vT.astype(np.float64))
    p_wst = float(w.astype(np.float64) @ T8.astype(np.float64)) + g0 \
        + float(b2.astype(np.float64) @ T8.astype(np.float64))
    st = p_wst + Qe + aQ + float(b2.astype(np.float64) @ e8.astype(np.float64))
    out = st.astype(np.float32) @ fc_w.T + fc_b
    return out.astype(np.float32)


# revision 3
# speedup vs baseline: 1.0583x; 1.0583x over previous
"""Trainium2 Bass kernel for nn_Kalman_filter_34041910788634.

Mathematical collapse of the reference:
  - The scan's step() ignores its carry (st, e_t = inp rebinds both from the
    scan inputs), and the parameter-network output o is time-invariant, so the
    whole T_LEN-step loop reduces to evaluating the last step (T[-1], e[-1]).
  - The second MLP matmul (h @ W2.T, 34 GFLOP) is only consumed through dot
    products with e8 and T8, so it collapses to h @ (W2.T @ e8) and
    h[0] @ (W2.T @ T8): two matvecs.

Device work per core k (hidden dim sharded 8 ways), all fp32r full-rate:
  hQ_k.T = relu(W1_k @ Q.T + b1_k)   [512, 2048]   (the one big matmul)
  aq_k   = ve_k.T @ hQ_k.T           [2048]        (partial over hidden shard)
Everything else (ve/vT/Qe/hw matvecs, final fc — ~50 MFLOP total vs 34 GFLOP)
is host-side glue around the sharded launch.

Layout choices: host passes W1_k.T and Q.T so both matmul operands load into
SBUF in their natural [contraction-on-partitions, free] layout, no on-device
transposes.  DMAs are issued in exact consumption order on two HWDGE rings
(w1t on the ACT ring, the Q.T stream on the SP ring) so the first matmul only
waits for two 256 KB tiles.  The dc-outer loop order lets the PE consume each
arriving Q.T tile with 4 matmuls immediately; each r-chunk's aq reduction is
delayed by one sweep so the PE never waits on ACT relus.
"""

import os
import sys

for _p in ("/opt/trn_rl_repo", "/root/.axon_site/_ro/trn_rl_repo"):
    if os.path.isdir(_p) and _p not in sys.path:
        sys.path.insert(0, _p)

import numpy as np

import concourse.bass as bass
import concourse.bass2jax as _bass2jax
import concourse.mybir as mybir
import concourse.tile as tile
from concourse.bass_utils import run_bass_kernel_spmd


def _split_multiwaits(bir_bytes):
    """The walrus build in this container supports at most one sync-wait
    condition per instruction; Tile freely emits several.  Hoist extra waits
    onto NoOp instructions inserted just before the owning instruction (same
    engine, so per-engine program order makes this equivalent)."""
    import orjson

    b = orjson.loads(bir_bytes)
    n = 0
    for func in b.get("functions", []):
        for blk in func.get("blocks", []):
            newl = []
            for ins in blk.get("instructions", []):
                si = ins.get("sync_info")
                ws = (si or {}).get("on_wait") or []
                if len(ws) > 1:
                    for wv in ws[:-1]:
                        n += 1
                        newl.append({
                            "debug": ins.get("debug", 0),
                            "engine": ins["engine"],
                            "ins": [],
                            "outs": [],
                            "name": f"{ins['name']}-wsplit{n}",
                            "opcode": "NoOp",
                            "sync_info": {"on_update": [], "on_wait": [wv]},
                        })
                    si["on_wait"] = ws[-1:]
                newl.append(ins)
            blk["instructions"] = newl
    return orjson.dumps(b)


_orig_compile_bir_kernel = _bass2jax.compile_bir_kernel


def _patched_compile_bir_kernel(ant_bir_str, compile_dir, neff_name="file.neff"):
    return _orig_compile_bir_kernel(
        _split_multiwaits(ant_bir_str), compile_dir, neff_name=neff_name
    )


if _bass2jax.compile_bir_kernel is not _patched_compile_bir_kernel:
    _bass2jax.compile_bir_kernel = _patched_compile_bir_kernel


N_DIM = 2048
HIDDEN = 4096
OUT_DIM = 512
NCORES = 8
JSH = HIDDEN // NCORES      # 512 hidden units per core
DC = N_DIM // 128           # 16 contraction chunks
JC = JSH // 128             # 4 lhsT column chunks
RC = N_DIM // 512           # 4 moving-dim chunks of 512

FR = mybir.dt.float32r
F32 = mybir.dt.float32
RELU = mybir.ActivationFunctionType.Relu

_cache = {}


def _build_nc():
    nc = bass.Bass(target_bir_lowering=False)

    qtr = nc.dram_tensor("qtr", [RC, N_DIM, 512], FR, kind="ExternalInput")
    w1t = nc.dram_tensor("w1t", [N_DIM, JSH], FR, kind="ExternalInput")
    vec = nc.dram_tensor("vec", [128, JC], FR, kind="ExternalInput")
    b1c = nc.dram_tensor("b1c", [128, JC], F32, kind="ExternalInput")
    aq = nc.dram_tensor("aq", [1, N_DIM], F32, kind="ExternalOutput")

    with tile.TileContext(nc) as tc:
        with (
            tc.tile_pool(name="wpool", bufs=1) as wpool,
            tc.tile_pool(name="qpool", bufs=2) as qpool,
            tc.tile_pool(name="small", bufs=1) as small,
            tc.tile_pool(name="hpool", bufs=8) as hpool,
            tc.tile_pool(name="opool", bufs=1) as opool,
            tc.tile_pool(name="psh", bufs=6, space="PSUM") as psh,
            tc.tile_pool(name="psv", bufs=2, space="PSUM") as psv,
        ):
            # Small, then weights on the ACT HWDGE ring (parallel to qt's SP
            # ring); both in consumption order.
            vec_s = small.tile([128, JC], FR, name="vec_s")
            nc.scalar.dma_start(vec_s[:], vec[:])
            b1c_s = small.tile([128, JC], F32, name="b1c_s")
            nc.scalar.dma_start(b1c_s[:], b1c[:])

            w1ts = []
            for dc in range(DC):
                t = wpool.tile([128, JSH], FR, name=f"w1t_{dc}", tag=f"w1t_{dc}")
                nc.scalar.dma_start(t[:], w1t[dc * 128:(dc + 1) * 128, :])
                w1ts.append(t)

            aq_s = opool.tile([1, N_DIM], F32, name="aq_s")

            hrelus = {}

            def emit_aq(r):
                pa = psv.tile([1, 512], F32, name=f"pa_{r}", tag="pa")
                for jc in range(JC):
                    nc.tensor.matmul(
                        pa[:],
                        vec_s[:, jc:jc + 1],
                        hrelus[(r, jc)][:],
                        start=(jc == 0),
                        stop=(jc == JC - 1),
                    )
                nc.vector.tensor_copy(aq_s[:, r * 512:(r + 1) * 512], pa[:])

            for rc in range(RC):
                qts = []
                for dc in range(DC):
                    t = qpool.tile([128, 512], FR, name=f"qt_{rc}_{dc}", tag=f"qt_{dc}")
                    nc.sync.dma_start(t[:], qtr[rc, dc * 128:(dc + 1) * 128, :])
                    qts.append(t)
                phs = [
                    psh.tile([128, 512], F32, name=f"ph_{rc}_{jc}", tag="ph")
                    for jc in range(JC)
                ]
                for dc in range(DC):
                    for jc in range(JC):
                        nc.tensor.matmul(
                            phs[jc][:],
                            w1ts[dc][:, jc * 128:(jc + 1) * 128],
                            qts[dc][:],
                            start=(dc == 0),
                            stop=(dc == DC - 1),
                        )
                for jc in range(JC):
                    hr = hpool.tile([128, 512], FR, name=f"hr_{rc}_{jc}", tag="hr")
                    nc.scalar.activation(hr[:], phs[jc][:], RELU,
                                         bias=b1c_s[:, jc:jc + 1])
                    hrelus[(rc, jc)] = hr
                if rc >= 1:
                    emit_aq(rc - 1)
            emit_aq(RC - 1)

            nc.sync.dma_start(aq[:], aq_s[:])

    return nc


def _get_nc():
    if "nc" not in _cache:
        _cache["nc"] = _build_nc()
    return _cache["nc"]


def _col128(v):
    """[n*128] -> [128, n] with v[c*128+p] at [p, c]."""
    return np.ascontiguousarray(v.reshape(-1, 128).T)


def kernel(**inputs):
    T = np.asarray(inputs["T"], np.float32)
    e = np.asarray(inputs["e"], np.float32)
    w = np.asarray(inputs["w"], np.float32)
    Q = np.asarray(inputs["Q"], np.float32)
    W1 = np.asarray(inputs["W1"], np.float32)
    b1 = np.asarray(inputs["b1"], np.float32)
    W2 = np.asarray(inputs["W2"], np.float32)
    b2 = np.asarray(inputs["b2"], np.float32)
    fc_w = np.asarray(inputs["fc_w"], np.float32)
    fc_b = np.asarray(inputs["fc_b"], np.float32)

    T8 = T[-1]
    e8 = e[-1]

    QT = np.ascontiguousarray(Q.T)                      # [d, r]
    qtr = np.ascontiguousarray(
        QT.reshape(N_DIM, RC, 512).transpose(1, 0, 2))  # [rc, d, 512]
    ve = e8 @ W2                                        # [4096] = W2.T @ e8
    vT = T8 @ W2

    in_maps = []
    for k in range(NCORES):
        in_maps.append({
            "qtr": qtr,
            "w1t": np.ascontiguousarray(W1[k * JSH:(k + 1) * JSH, :].T),
            "vec": _col128(ve[k * JSH:(k + 1) * JSH]),
            "b1c": _col128(b1[k * JSH:(k + 1) * JSH]),
        })

    res = run_bass_kernel_spmd(_get_nc(), in_maps, core_ids=list(range(NCORES))).results

    aQ = np.zeros(N_DIM, np.float64)
    for k in range(NCORES):
        aQ += res[k]["aq"][0].astype(np.float64)

    # Host-side glue (tiny BLAS-1/2): Qe, hw row, scalars, final fc.
    Qe = (Q.astype(np.float64) @ e8.astype(np.float64))
    hw = np.maximum(W1.astype(np.float64) @ w.astype(np.float64)
                    + b1.astype(np.float64), 0.0)
    g0 = float(hw @ vT.astype(np.float64))
    p_wst = float(w.astype(np.float64) @ T8.astype(np.float64)) + g0 \
        + float(b2.astype(np.float64) @ T8.astype(np.float64))
    st = p_wst + Qe + aQ + float(b2.astype(np.float64) @ e8.astype(np.float64))
    out = st.astype(np.float32) @ fc_w.T + fc_b
    return out.astype(np.float32)


# revision 6
# speedup vs baseline: 1.1081x; 1.0471x over previous
"""Trainium2 Bass kernel for nn_Kalman_filter_34041910788634.

Mathematical collapse of the reference:
  - The scan's step() ignores its carry (st, e_t = inp rebinds both from the
    scan inputs), and the parameter-network output o is time-invariant, so the
    whole T_LEN-step loop reduces to evaluating the last step (T[-1], e[-1]).
  - The second MLP matmul (h @ W2.T, 34 GFLOP) is only consumed through dot
    products with e8 and T8, so it collapses to h @ (W2.T @ e8) and
    h[0] @ (W2.T @ T8): two matvecs.

Device work per core k (hidden dim sharded 8 ways), all fp32r full-rate:
  hQ_k.T = relu(W1_k @ Q.T + b1_k)   [512, 2048]   (the one big matmul)
  aq_k   = ve_k.T @ hQ_k.T           [2048]        (partial over hidden shard)
Everything else (ve/vT/Qe/hw matvecs, final fc — ~50 MFLOP total vs 34 GFLOP)
is host-side glue around the sharded launch.

Layout choices: host passes W1_k.T and Q.T so both matmul operands load into
SBUF in their natural [contraction-on-partitions, free] layout, no on-device
transposes.  DMAs are issued in exact consumption order on two HWDGE rings
(w1t on the ACT ring, the Q.T stream on the SP ring) so the first matmul only
waits for two 256 KB tiles.  The dc-outer loop order lets the PE consume each
arriving Q.T tile with 4 matmuls immediately; each r-chunk's aq reduction is
delayed by one sweep so the PE never waits on ACT relus.
"""

import os
import sys

for _p in ("/opt/trn_rl_repo", "/root/.axon_site/_ro/trn_rl_repo"):
    if os.path.isdir(_p) and _p not in sys.path:
        sys.path.insert(0, _p)

import numpy as np

import concourse.bass as bass
import concourse.bass2jax as _bass2jax
import concourse.mybir as mybir
import concourse.tile as tile
from concourse.bass_utils import run_bass_kernel_spmd


def _split_multiwaits(bir_bytes):
    """The walrus build in this container supports at most one sync-wait
    condition per instruction; Tile freely emits several.  Hoist extra waits
    onto NoOp instructions inserted just before the owning instruction (same
    engine, so per-engine program order makes this equivalent)."""
    import orjson

    b = orjson.loads(bir_bytes)
    n = 0
    for func in b.get("functions", []):
        for blk in func.get("blocks", []):
            newl = []
            for ins in blk.get("instructions", []):
                si = ins.get("sync_info")
                ws = (si or {}).get("on_wait") or []
                if len(ws) > 1:
                    for wv in ws[:-1]:
                        n += 1
                        newl.append({
                            "debug": ins.get("debug", 0),
                            "engine": ins["engine"],
                            "ins": [],
                            "outs": [],
                            "name": f"{ins['name']}-wsplit{n}",
                            "opcode": "NoOp",
                            "sync_info": {"on_update": [], "on_wait": [wv]},
                        })
                    si["on_wait"] = ws[-1:]
                newl.append(ins)
            blk["instructions"] = newl
    return orjson.dumps(b)


_orig_compile_bir_kernel = _bass2jax.compile_bir_kernel


def _patched_compile_bir_kernel(ant_bir_str, compile_dir, neff_name="file.neff"):
    return _orig_compile_bir_kernel(
        _split_multiwaits(ant_bir_str), compile_dir, neff_name=neff_name
    )


if _bass2jax.compile_bir_kernel is not _patched_compile_bir_kernel:
    _bass2jax.compile_bir_kernel = _patched_compile_bir_kernel


N_DIM = 2048
HIDDEN = 4096
OUT_DIM = 512
NCORES = 8
JSH = HIDDEN // NCORES      # 512 hidden units per core
DC = N_DIM // 128           # 16 contraction chunks
JC = JSH // 128             # 4 lhsT column chunks
RC = N_DIM // 512           # 4 moving-dim chunks of 512

FR = mybir.dt.float32r
BF = mybir.dt.bfloat16
F32 = mybir.dt.float32
RELU = mybir.ActivationFunctionType.Relu

_cache = {}


def _build_nc():
    nc = bass.Bass(target_bir_lowering=False)

    qtr = nc.dram_tensor("qtr", [RC, N_DIM, 512], FR, kind="ExternalInput")
    w1t = nc.dram_tensor("w1t", [N_DIM, JSH], FR, kind="ExternalInput")
    vec = nc.dram_tensor("vec", [128, JC], FR, kind="ExternalInput")
    b1c = nc.dram_tensor("b1c", [128, JC], F32, kind="ExternalInput")
    aq = nc.dram_tensor("aq", [1, N_DIM], F32, kind="ExternalOutput")

    with tile.TileContext(nc) as tc:
        with (
            tc.tile_pool(name="wpool", bufs=1) as wpool,
            tc.tile_pool(name="qpool", bufs=3) as qpool,
            tc.tile_pool(name="small", bufs=1) as small,
            tc.tile_pool(name="hpool", bufs=8) as hpool,
            tc.tile_pool(name="opool", bufs=1) as opool,
            tc.tile_pool(name="psh", bufs=6, space="PSUM") as psh,
            tc.tile_pool(name="psv", bufs=2, space="PSUM") as psv,
        ):
            # Small, then weights on the ACT HWDGE ring (parallel to qt's SP
            # ring); both in consumption order.
            vec_s = small.tile([128, JC], FR, name="vec_s")
            nc.scalar.dma_start(vec_s[:], vec[:])
            b1c_s = small.tile([128, JC], F32, name="b1c_s")
            nc.scalar.dma_start(b1c_s[:], b1c[:])

            w1ts = []
            for dc in range(DC):
                t = wpool.tile([128, JSH], FR, name=f"w1t_{dc}", tag=f"w1t_{dc}")
                nc.scalar.dma_start(t[:], w1t[dc * 128:(dc + 1) * 128, :])
                w1ts.append(t)

            aq_s = opool.tile([1, N_DIM], F32, name="aq_s")

            hrelus = {}

            def emit_aq(r):
                pa = psv.tile([1, 512], F32, name=f"pa_{r}", tag="pa")
                for jc in range(JC):
                    nc.tensor.matmul(
                        pa[:],
                        vec_s[:, jc:jc + 1],
                        hrelus[(r, jc)][:],
                        start=(jc == 0),
                        stop=(jc == JC - 1),
                    )
                nc.vector.tensor_copy(aq_s[:, r * 512:(r + 1) * 512], pa[:])
                nc.sync.dma_start(aq[:, r * 512:(r + 1) * 512],
                                  aq_s[:, r * 512:(r + 1) * 512])

            for rc in range(RC):
                qts = []
                for dc in range(DC):
                    t = qpool.tile([128, 512], FR, name=f"qt_{rc}_{dc}", tag=f"qt_{dc}")
                    nc.sync.dma_start(t[:], qtr[rc, dc * 128:(dc + 1) * 128, :])
                    qts.append(t)
                phs = [
                    psh.tile([128, 512], F32, name=f"ph_{rc}_{jc}", tag="ph")
                    for jc in range(JC)
                ]
                for dc in range(DC):
                    for jc in range(JC):
                        nc.tensor.matmul(
                            phs[jc][:],
                            w1ts[dc][:, jc * 128:(jc + 1) * 128],
                            qts[dc][:],
                            start=(dc == 0),
                            stop=(dc == DC - 1),
                        )
                for jc in range(JC):
                    hr = hpool.tile([128, 512], FR, name=f"hr_{rc}_{jc}", tag="hr")
                    nc.scalar.activation(hr[:], phs[jc][:], RELU,
                                         bias=b1c_s[:, jc:jc + 1])
                    hrelus[(rc, jc)] = hr
                if rc >= 1:
                    emit_aq(rc - 1)
            emit_aq(RC - 1)

    return nc


def _get_nc():
    if "nc" not in _cache:
        _cache["nc"] = _build_nc()
    return _cache["nc"]


def _col128(v):
    """[n*128] -> [128, n] with v[c*128+p] at [p, c]."""
    return np.ascontiguousarray(v.reshape(-1, 128).T)


def kernel(**inputs):
    T = np.asarray(inputs["T"], np.float32)
    e = np.asarray(inputs["e"], np.float32)
    w = np.asarray(inputs["w"], np.float32)
    Q = np.asarray(inputs["Q"], np.float32)
    W1 = np.asarray(inputs["W1"], np.float32)
    b1 = np.asarray(inputs["b1"], np.float32)
    W2 = np.asarray(inputs["W2"], np.float32)
    b2 = np.asarray(inputs["b2"], np.float32)
    fc_w = np.asarray(inputs["fc_w"], np.float32)
    fc_b = np.asarray(inputs["fc_b"], np.float32)

    T8 = T[-1]
    e8 = e[-1]

    QT = np.ascontiguousarray(Q.T)                      # [d, r]
    qtr = np.ascontiguousarray(
        QT.reshape(N_DIM, RC, 512).transpose(1, 0, 2))  # [rc, d, 512]
    ve = e8 @ W2                                        # [4096] = W2.T @ e8
    vT = T8 @ W2

    in_maps = []
    for k in range(NCORES):
        in_maps.append({
            "qtr": qtr,
            "w1t": np.ascontiguousarray(W1[k * JSH:(k + 1) * JSH, :].T),
            "vec": _col128(ve[k * JSH:(k + 1) * JSH]),
            "b1c": _col128(b1[k * JSH:(k + 1) * JSH]),
        })

    res = run_bass_kernel_spmd(_get_nc(), in_maps, core_ids=list(range(NCORES))).results

    aQ = np.zeros(N_DIM, np.float64)
    for k in range(NCORES):
        aQ += res[k]["aq"][0].astype(np.float64)

    # Host-side glue (tiny BLAS-1/2): Qe, hw row, scalars, final fc.
    Qe = (Q.astype(np.float64) @ e8.astype(np.float64))
    hw = np.maximum(W1.astype(np.float64) @ w.astype(np.float64)
                    + b1.astype(np.float64), 0.0)
    g0 = float(hw @ vT.astype(np.float64))
    p_wst = float(w.astype(np.float64) @ T8.astype(np.float64)) + g0 \
        + float(b2.astype(np.float64) @ T8.astype(np.float64))
    st = p_wst + Qe + aQ + float(b2.astype(np.float64) @ e8.astype(np.float64))
    out = st.astype(np.float32) @ fc_w.T + fc_b
    return out.astype(np.float32)
